# revision 44
# baseline (speedup 1.0000x reference)
"""Trainium2 Bass kernel for nn_DecoderRNN (LSTM decoder + 32k-vocab projection).

Distribution over 8 NeuronCores:
  - The 23-step LSTM recurrence (B=64) is replicated on every core. Per step a
    fused fp16 weight stream [w | cS | hS | cU | hU | ones] (1602 rows in 13
    128-row chunks) x [lstmS gates | lstmU gates] (2600 cols) accumulates in
    fp32 PSUM. Gate columns are interleaved host-side as [i|f|o|g] x 125
    features per 500-col PSUM bank so one sigmoid covers i,f,o and one tanh
    covers g (2 activation instructions per bank instead of 4).
  - All recurrent state (cS/hS/cU/hU) is produced and kept in fp16: the
    DVE state-update ops write fp16 directly, so the per-step PE transposes
    run at 1 cycle/col instead of fp32's 2 (fp32 transposes were ~14% of
    tensor-engine time) and every state copy moves half the bytes.
  - The vocab projection (61 GFLOP) is sharded 8-way over vocab: each core
    keeps a [626, 4000] bf16 fcW slice (incl. fcb row) resident in SBUF and
    computes [24*64, 626] @ [626, 4000]. The projection is split into 96
    (m,n) units of five 500-col matmuls; units are interleaved into every
    recurrence step behind the bank matmuls but ahead of the (PE) state
    transposes, so the PE streams continuously and its DVFS ramp never
    resets (a cold tensor engine runs at half clock for ~4us).
  - wordOut staging tiles are split per fc m-chunk so a unit's read depends
    only on its own scatter writes, not the latest step's.
  - make_ft (64 batched 25x25 matmuls/step) is software-pipelined one step
    behind the recurrence so its DRAM-bounce regroup DMAs are off the PE
    critical path.
  - Word embeddings are gathered/transposed on the host (indexing only).
"""

import numpy as np
import ml_dtypes

B, T, V, E, D = 64, 24, 32000, 256, 25
H = D * D                       # 625
GCOLS = 4 * H + 4 * D           # 2600
NCORES = 8
NV = 8                          # vocab shards (batch replicated)
VL = V // NV                    # 4000
NM = T // 2                     # 12 fc m-chunks of 128 rows (2 time cols)
NU = VL // 500                  # 8 n-units per m-chunk
NSTEP = T - 1                   # 23
F16, BF16 = np.float16, ml_dtypes.bfloat16

# x-row layout: every block 32-aligned within its 128-chunk (pad rows carry
# zero weights, pad state columns are zeroed)
XW = 0            # 256 word rows
XCS = 256         # 625 cS rows, padded to 640
XHS = 896         # 625 hS rows, padded to 640
XCU = 1536        # 25 cU rows, padded to 32
XHU = 1568        # 25 hU rows, padded to 32
XONE = 1600       # 2 ones rows
XROWS = 1602
NCHUNK = 13       # 12*128 + 66

_COMPILED = {}


def _chunk_rows(c):
    return 128 if c < NCHUNK - 1 else XROWS - 128 * (NCHUNK - 1)


def _win(start, end):
    out, p = [], start
    while p < end:
        n = min(end - p, 128 * (p // 128 + 1) - p)
        out.append((p, n))
        p += n
    return out


def _build_program():
    import concourse.bass as bass
    import concourse.tile as tile
    from concourse import bacc, mybir
    from contextlib import ExitStack

    f16, f32 = mybir.dt.float16, mybir.dt.float32
    bf16 = mybir.dt.bfloat16
    AF = mybir.ActivationFunctionType

    nc = bacc.Bacc("TRN2", target_bir_lowering=False, debug=False,
                   num_devices=NCORES)

    w_all = nc.dram_tensor("w_all", [XROWS, GCOLS], f16, kind="ExternalInput").ap()
    fcq = nc.dram_tensor("fcq", [H + 1, VL], bf16, kind="ExternalInput").ap()
    wut = nc.dram_tensor("wut", [34, H], f16, kind="ExternalInput").ap()
    feat = nc.dram_tensor("feat", [E + 1, B], f16, kind="ExternalInput").ap()
    szt = nc.dram_tensor("szt", [E + 1, H], f16, kind="ExternalInput").ap()
    wt0 = nc.dram_tensor("wt0", [128, 1536], f16, kind="ExternalInput").ap()
    wt1 = nc.dram_tensor("wt1", [128, 1536], f16, kind="ExternalInput").ap()
    iden = nc.dram_tensor("iden", [128, 128], f16, kind="ExternalInput").ap()
    onecol = nc.dram_tensor("onecol", [64, 64], f16, kind="ExternalInput").ap()
    ones16 = nc.dram_tensor("ones16", [2, 64], f16, kind="ExternalInput").ap()
    onesbf = nc.dram_tensor("onesbf", [1, 768], bf16, kind="ExternalInput").ap()
    zerbf = nc.dram_tensor("zerbf", [126, 64], bf16, kind="ExternalInput").ap()
    out = nc.dram_tensor("out", [T, B, VL], bf16, kind="ExternalOutput").ap()

    with tile.TileContext(nc) as tc, ExitStack() as ctx:
        const = ctx.enter_context(tc.tile_pool(name="const", bufs=1))
        states = ctx.enter_context(tc.tile_pool(name="states", bufs=3))
        gact = ctx.enter_context(tc.tile_pool(name="gact", bufs=1))
        xts_pool = ctx.enter_context(tc.tile_pool(name="xtsp", bufs=3))
        ft_pool = ctx.enter_context(tc.tile_pool(name="ftp", bufs=3))
        ft1_pool = ctx.enter_context(tc.tile_pool(name="ftp1", bufs=1))
        lout = ctx.enter_context(tc.tile_pool(name="loutp", bufs=12))
        dram = ctx.enter_context(tc.tile_pool(name="dram", bufs=6, space="DRAM"))
        gpsum = ctx.enter_context(tc.tile_pool(name="gpsum", bufs=4, space="PSUM"))
        tpsum = ctx.enter_context(tc.tile_pool(name="tpsum", bufs=2, space="PSUM"))
        fcpsum = ctx.enter_context(tc.tile_pool(name="fcps", bufs=2, space="PSUM"))

        # ---------- persistent SBUF (init-critical small tensors first so
        # the sStart matmul + first transposes aren't stuck behind the big
        # weight stream; big loads alternate between the two HWDGE queues) --
        FEAT, SZT = [], []
        for c, r in ((0, 128), (1, 128), (2, 1)):
            tf = const.tile([r, B], f16, tag=f"feat{c}")
            nc.sync.dma_start(tf[:], feat[128 * c:128 * c + r, :])
            FEAT.append(tf)
            ts = const.tile([r, H], f16, tag=f"szt{c}")
            nc.scalar.dma_start(ts[:], szt[128 * c:128 * c + r, :])
            SZT.append(ts)
        IDEN = const.tile([128, 128], f16, tag="iden")
        nc.sync.dma_start(IDEN[:], iden)
        ONECOL = const.tile([64, 64], f16, tag="onecol")
        nc.sync.dma_start(ONECOL[:], onecol)
        ONES16 = const.tile([2, 64], f16, tag="ones16")
        nc.sync.dma_start(ONES16[:], ones16)
        WUT = const.tile([34, H], f16, tag="wut")
        nc.scalar.dma_start(WUT[:], wut)
        WT0 = const.tile([128, 1536], f16, tag="wt0")
        nc.sync.dma_start(WT0[:], wt0)
        WT1 = const.tile([128, 1536], f16, tag="wt1")
        nc.scalar.dma_start(WT1[:], wt1)
        WA = []
        for c in range(NCHUNK):
            r = _chunk_rows(c)
            t_ = const.tile([r, GCOLS], f16, tag=f"wa{c}")
            eng = nc.sync if c % 2 == 0 else nc.scalar
            eng.dma_start(t_[:], w_all[128 * c:128 * c + r, :])
            WA.append(t_)
        F16Z = const.tile([64, 16], f16, tag="f16z")
        nc.vector.memset(F16Z[:], 0.0)

        # wordOut staging, one tile set per fc m-chunk (time cols 2m, 2m+1)
        WO = []
        for m in range(NM):
            row = []
            for c in range(5):
                r = 126 if c == 4 else 125
                t_ = const.tile([r, 128], bf16, tag=f"wo{m}_{c}")
                if m == 0:
                    nc.sync.dma_start(t_[:, 0:B], zerbf[0:r, :])
                row.append(t_)
            WO.append(row)
        for m in range(NM):
            nc.sync.dma_start(WO[m][4][125:126, :], onesbf[:, 0:128])
        FC = []
        for c in range(5):
            r = 126 if c == 4 else 125
            t_ = const.tile([r, VL], bf16, tag=f"fc{c}")
            eng = nc.sync if c % 2 == 0 else nc.scalar
            eng.dma_start(t_[:], fcq[125 * c:125 * c + r, :])
            FC.append(t_)

        # ---------- helpers ----------
        def xts_ap(xts, xr0, n):
            c = xr0 // 128
            assert (xr0 + n - 1) // 128 == c and c >= 2
            return xts[xr0 % 128:xr0 % 128 + n,
                       64 * (c - 2):64 * (c - 2) + 64]

        def transpose_into(src_, xts, xr0, total):
            col = 0
            for (p0, n) in _win(xr0, xr0 + total):
                ps = tpsum.tile([128, B], f16, tag="tp")
                nc.tensor.transpose(ps[0:n, :], src_[:, col:col + n],
                                    IDEN[0:B, 0:B])
                nc.vector.tensor_copy(xts_ap(xts, p0, n), ps[0:n, :])
                col += n

        BANKS = [(0, 500), (500, 500), (1000, 500), (1500, 500),
                 (2000, 500), (2500, 100)]

        # fc work split into (m, n) units of 5 accumulating 500-col matmuls
        # each (~1us at full clock), emitted singly into recurrence stall
        # slots. m-chunk m (time cols 2m, 2m+1) needs ft(tcol 2m+1) which is
        # scattered by emit_ft_matmuls(2m) during step 2m+2 -> legal s>=2m+3.
        fc_cursor = [0]
        NPAIR = NU // 2

        def emit_fc_units(k):
            # pair-units: the WO stationary is reused across two 500-col
            # matmuls so its LDWEIGHTS amortizes
            for _ in range(k):
                u = fc_cursor[0]
                if u >= NM * NPAIR:
                    return
                fc_cursor[0] += 1
                m, pr = u // NPAIR, u % NPAIR
                fp0 = fcpsum.tile([128, 500], f32, tag="fcp")
                fp1 = fcpsum.tile([128, 500], f32, tag="fcp")
                fps = [fp0, fp1]
                for c in range(5):
                    for j in range(2):
                        n = 2 * pr + j
                        nc.tensor.matmul(
                            fps[j][:], WO[m][c][:],
                            FC[c][:, 500 * n:500 * n + 500],
                            start=(c == 0), stop=(c == 4),
                            skip_group_check=True)
                for j in range(2):
                    n = 2 * pr + j
                    lo = lout.tile([128, 500], bf16, tag="lo")
                    if j == 0:
                        nc.vector.tensor_copy(lo[:], fps[j][:])
                    else:
                        nc.scalar.activation(lo[:], fps[j][:], AF.Copy)
                    nc.sync.dma_start(
                        out[2 * m:2 * m + 2, :, 500 * n:500 * n + 500], lo[:])

        # ---------- initial state (all recurrent state is fp16) ----------
        CS = states.tile([B, 640], f16, tag="cs")
        sp0 = gpsum.tile([B, 500], f32, tag="gp")
        sp1 = gpsum.tile([B, 500], f32, tag="gp")
        for ci in range(3):
            nc.tensor.matmul(sp0[:, 0:500], FEAT[ci][:], SZT[ci][:, 0:500],
                             start=(ci == 0), stop=(ci == 2))
            nc.tensor.matmul(sp1[:, 0:125], FEAT[ci][:], SZT[ci][:, 500:625],
                             start=(ci == 0), stop=(ci == 2))
        nc.vector.tensor_copy(CS[:, 0:500], sp0[:, 0:500])
        nc.vector.tensor_copy(CS[:, 500:625], sp1[:, 0:125])
        nc.vector.tensor_copy(CS[:, 625:640], F16Z[:, 0:15])
        CU = states.tile([B, 32], f16, tag="cu")
        nc.vector.memset(CU[:], 0.0)

        XTS = xts_pool.tile([128, 11 * 64], f16, tag="xts")
        nc.vector.memset(XTS[:], 0.0)
        nc.vector.tensor_copy(xts_ap(XTS, XONE, 2), ONES16[:])
        transpose_into(CS, XTS, XCS, 640)

        # make_ft tiles carried across two steps (software pipelining deep
        # enough that the DRAM-bounce regroup DMAs always land early)
        pipe = []

        def emit_ft_matmuls(s):
            # consume UTT/M2T (DMA'd during step s) -> TTS -> ttd -> WO col s+1
            UTT, M2T = pipe.pop(0)
            TTS = ft1_pool.tile([D, B * D], f32, tag="tts")
            for q in range(4):
                ttf = gpsum.tile([B, 500], f32, tag="gp")
                tt = ttf[0:D, 0:16 * D]
                for bl in range(16):
                    b = 16 * q + bl
                    nc.tensor.matmul(tt[:, D * bl:D * bl + D],
                                     M2T[:, D * b:D * b + D],
                                     UTT[:, D * b:D * b + D],
                                     start=True, stop=True)
                tdst = (TTS[:].rearrange("j (i b) -> j i b", i=D)
                        [:, :, 16 * q:16 * q + 16])
                tsrc = tt[:].rearrange("j (b i) -> j i b", b=16)
                if q % 2 == 0:
                    nc.vector.tensor_copy(tdst, tsrc)
                else:
                    nc.scalar.activation(tdst, tsrc, AF.Copy)
            ttd = dram.tile([D, B * D], f32, tag="ttd")
            nc.sync.dma_start(ttd[:], TTS[:])
            tcol = s + 1
            wm, wl = tcol // 2, tcol % 2
            for c in range(5):
                nc.gpsimd.dma_start(
                    WO[wm][c][0:125, B * wl:B * wl + B],
                    bass.AP(ttd.tensor, 5 * c * B * D,
                            [[B * D, 5], [B, D], [1, B]]))

        # ---------- recurrence ----------
        for s in range(NSTEP):
            def xchunk(c, _xts=XTS, _s=s):
                if c == 0:
                    return WT0[:, 64 * _s:64 * _s + 64]
                if c == 1:
                    return WT1[:, 64 * _s:64 * _s + 64]
                r = _chunk_rows(c)
                return _xts[0:r, 64 * (c - 2):64 * (c - 2) + 64]

            gps = []
            for (col0, n) in BANKS:
                gp = gpsum.tile([B, 500], f32, tag="gp")
                # the U-gate bank (cols 2500:) has structurally zero weights
                # for the hS row chunks 7..11 — skip those matmuls
                chunks = (list(range(NCHUNK)) if col0 < 4 * H
                          else [0, 1, 2, 3, 4, 5, 6, 12])
                for j, c in enumerate(chunks):
                    nc.tensor.matmul(gp[:, 0:n], xchunk(c),
                                     WA[c][:, col0:col0 + n],
                                     start=(j == 0), stop=(j == len(chunks) - 1))
                gps.append(gp)

            # PE fill work while the step-s activation/update tail runs
            if s >= 2:
                emit_ft_matmuls(s - 2)

            # fc units behind the banks but ahead of the (PE) transposes:
            # they stream while Scalar/Vector run this step's act+update, so
            # the PE never idles and its DVFS ramp never resets.
            allowed = (NPAIR * ((s - 3) // 2 + 1)) if s >= 3 else 0
            emit_fc_units(min(max(allowed - fc_cursor[0], 0), 2))

            # per-bank [i|f|o|g] x 125-feature activation + state update,
            # cS'/hS' written directly as fp16
            CSn = states.tile([B, 640], f16, tag="cs")
            HSn = states.tile([B, 640], f16, tag="hs")
            TC = gact.tile([B, H], f32, tag="tc")
            for b in range(5):
                fsl = slice(125 * b, 125 * b + 125)
                sio = gact.tile([B, 375], f32, tag="gsio")
                gb = gact.tile([B, 125], f32, tag="ggb")
                nc.scalar.activation(sio[:], gps[b][:, 0:375], AF.Sigmoid)
                nc.scalar.activation(gb[:], gps[b][:, 375:500], AF.Tanh)
                t1 = gact.tile([B, 125], f32, tag="t1")
                t2 = gact.tile([B, 125], f32, tag="t2")
                nc.vector.tensor_mul(t1[:], sio[:, 125:250], CS[:, fsl])
                nc.vector.tensor_mul(t2[:], sio[:, 0:125], gb[:])
                nc.vector.tensor_add(CSn[:, fsl], t1[:], t2[:])
                nc.scalar.activation(TC[:, fsl], CSn[:, fsl], AF.Tanh)
                nc.vector.tensor_mul(HSn[:, fsl], sio[:, 250:375], TC[:, fsl])
            nc.vector.tensor_copy(CSn[:, H:640], F16Z[:, 0:15])
            nc.vector.tensor_copy(HSn[:, H:640], F16Z[:, 0:15])

            # U-LSTM (bank 5), gates laid out [i|f|o|g] x 25
            usio = gact.tile([B, 75], f32, tag="usio")
            ug = gact.tile([B, D], f32, tag="ug")
            nc.scalar.activation(usio[:], gps[5][:, 0:75], AF.Sigmoid)
            nc.scalar.activation(ug[:], gps[5][:, 75:100], AF.Tanh)
            CUn = states.tile([B, 32], f16, tag="cu")
            t1u = gact.tile([B, D], f32, tag="t1u")
            t2u = gact.tile([B, D], f32, tag="t2u")
            nc.vector.tensor_mul(t1u[:], usio[:, 25:50], CU[:, 0:D])
            nc.vector.tensor_mul(t2u[:], usio[:, 0:25], ug[:])
            nc.vector.tensor_add(CUn[:, 0:D], t1u[:], t2u[:])
            nc.vector.tensor_copy(CUn[:, D:32], F16Z[:, 0:7])
            TCU = gact.tile([B, D], f32, tag="tcu")
            nc.scalar.activation(TCU[:], CUn[:, 0:D], AF.Tanh)
            HUn = states.tile([B, 32], f16, tag="hu")
            nc.vector.tensor_mul(HUn[:, 0:D], usio[:, 50:75], TCU[:])
            nc.vector.tensor_copy(HUn[:, D:32], F16Z[:, 0:7])

            if s < NSTEP - 1:
                XTSn = xts_pool.tile([128, 11 * 64], f16, tag="xts")
                nc.vector.tensor_copy(xts_ap(XTSn, XONE, 2), ONES16[:])
                transpose_into(CSn, XTSn, XCS, 640)
                transpose_into(HSn, XTSn, XHS, 640)
            else:
                XTSn = None
            hups = tpsum.tile([128, B], f16, tag="tp")
            nc.tensor.transpose(hups[0:32, :], HUn[:], IDEN[0:B, 0:B])
            if XTSn is not None:
                transpose_into(CUn, XTSn, XCU, 32)
                nc.vector.tensor_copy(xts_ap(XTSn, XHU, 32), hups[0:32, :])
            HUTn = states.tile([34, B], f16, tag="hut")
            nc.vector.tensor_copy(HUTn[0:32, :], hups[0:32, :])
            nc.vector.tensor_copy(HUTn[32:34, :], ONES16[:])

            up0 = gpsum.tile([B, 500], f32, tag="gp")
            up1 = gpsum.tile([B, 500], f32, tag="gp")
            nc.tensor.matmul(up0[:, 0:500], HUTn[:], WUT[:, 0:500],
                             start=True, stop=True)
            nc.tensor.matmul(up1[:, 0:125], HUTn[:], WUT[:, 500:625],
                             start=True, stop=True)
            UT = ft1_pool.tile([B, H], f16, tag="ut")
            nc.vector.tensor_copy(UT[:, 0:500], up0[:, 0:500])
            nc.vector.tensor_copy(UT[:, 500:625], up1[:, 0:125])

            # stage make_ft inputs for all 64 batch rows via DRAM-bounce
            # regroup; consumed by emit_ft_matmuls at step s+2
            utd = dram.tile([B, H], f16, tag="utd")
            nc.sync.dma_start(utd[:], UT[:])
            m2d = dram.tile([B, H], f16, tag="m2d")
            nc.gpsimd.dma_start(m2d[:], HSn[:, 0:H])
            UTT = ft_pool.tile([D, B * D], f16, tag="utt")
            nc.sync.dma_start(
                UTT[:], bass.AP(utd.tensor, 0, [[D, D], [H, B], [1, D]]))
            M2T = ft_pool.tile([D, B * D], f16, tag="m2t")
            nc.sync.dma_start(
                M2T[:], bass.AP(m2d.tensor, 0, [[D, D], [H, B], [1, D]]))
            pipe.append((UTT, M2T))

            CS, CU, XTS = CSn, CUn, XTSn

            # one more unit at step end to cover the XTS-copy dependency of
            # the next step's first bank matmul
            emit_fc_units(min(max(allowed - fc_cursor[0], 0), 1))

        emit_ft_matmuls(NSTEP - 2)
        emit_ft_matmuls(NSTEP - 1)
        emit_fc_units(NM * NPAIR - fc_cursor[0])

    nc.compile()
    return nc


# interleaved S-gate column permutation: bank b (0..4) holds
# [i|f|o|g] x features 125b..125b+125
def _gate_perm():
    idx = np.empty(2500, np.int64)
    for p, g in enumerate((0, 1, 3, 2)):
        for b in range(5):
            idx[500 * b + 125 * p:500 * b + 125 * p + 125] = \
                625 * g + 125 * b + np.arange(125)
    return idx


# U gate columns reordered [i|f|o|g] x 25
def _ugate_perm():
    return np.concatenate([np.arange(25), 25 + np.arange(25),
                           75 + np.arange(25), 50 + np.arange(25)])


def _host_prep(inputs):
    f32 = lambda k: np.asarray(inputs[k], dtype=np.float32)
    features = f32("features")
    captions = np.asarray(inputs["captions"]).astype(np.int64)
    embed = f32("embed_table")
    WihS, WhhS = f32("WihS"), f32("WhhS")
    bihS, bhhS = f32("bihS"), f32("bhhS")
    WihU, WhhU = f32("WihU"), f32("WhhU")
    bihU, bhhU = f32("bihU"), f32("bhhU")
    fcW, fcb = f32("fcW"), f32("fcb")
    szW, szb = f32("szW"), f32("szb")
    wuW, wub = f32("wuW"), f32("wub")

    w_all = np.zeros((XROWS, GCOLS), np.float32)
    WihS_T, WihU_T = WihS.T, WihU.T
    w_all[XW:XW + 256, :2500] = WihS_T[25:281]
    w_all[XW:XW + 256, 2500:] = WihU_T[25:281]
    w_all[XCS:XCS + 625, :2500] = WihS_T[281:906]
    w_all[XCS:XCS + 625, 2500:] = WihU_T[281:906]
    w_all[XHS:XHS + 625, :2500] = WhhS.T
    w_all[XCU:XCU + 25, :2500] = WihS_T[0:25]
    w_all[XCU:XCU + 25, 2500:] = WihU_T[0:25]
    w_all[XHU:XHU + 25, 2500:] = WhhU.T
    w_all[XONE, :2500] = bihS + bhhS
    w_all[XONE, 2500:] = bihU + bhhU
    w_all[:, :2500] = w_all[:, _gate_perm()]
    w_all[:, 2500:] = w_all[:, 2500 + _ugate_perm()]
    w_all = np.ascontiguousarray(w_all).astype(F16)

    fcW_perm = np.ascontiguousarray(
        fcW.reshape(V, D, D).transpose(0, 2, 1).reshape(V, H))
    wuW_perm = np.ascontiguousarray(
        wuW.reshape(D, D, D).transpose(1, 0, 2).reshape(H, D))
    wub_perm = np.ascontiguousarray(wub.reshape(D, D).T.reshape(H))
    wut = np.zeros((34, H), np.float32)
    wut[0:25] = wuW_perm.T
    wut[32] = wub_perm
    wut = wut.astype(F16)

    szt = np.concatenate([szW.T, szb[None, :]], 0).astype(F16)
    emb16 = embed.astype(F16)
    iden = np.eye(128, dtype=F16)
    onecol = np.zeros((64, 64), F16)
    onecol[:, 0] = 1.0

    feat_r = np.concatenate([features.T,
                             np.ones((1, B), np.float32)], 0).astype(F16)
    # host-side embedding gather+transpose: wt[e, 64*s+b] = emb[cap[b,s], e]
    # slot 0 is the word0 start token = embed_table[0] for every batch row
    wt = np.zeros((E, 1536), F16)
    wt[:, 0:64] = emb16[0][:, None]
    for s in range(1, 23):
        wt[:, 64 * s:64 * s + 64] = emb16[captions[:, s]].T

    in_maps = []
    for vq in range(NV):
        fcq = np.concatenate(
            [fcW_perm.T[:, VL * vq:VL * vq + VL],
             fcb[None, VL * vq:VL * vq + VL]], 0).astype(BF16)
        in_maps.append({
            "w_all": w_all, "fcq": np.ascontiguousarray(fcq),
            "wut": wut, "feat": feat_r, "szt": szt,
            "wt0": np.ascontiguousarray(wt[0:128]),
            "wt1": np.ascontiguousarray(wt[128:256]),
            "iden": iden, "onecol": onecol,
            "ones16": np.ones((2, 64), F16),
            "onesbf": np.ones((1, 768), BF16),
            "zerbf": np.zeros((126, 64), BF16),
        })
    return in_maps


def kernel(**inputs):
    from concourse.bass_utils import run_bass_kernel_spmd

    if "prog" not in _COMPILED:
        _COMPILED["prog"] = _build_program()
    nc = _COMPILED["prog"]

    in_maps = _host_prep(inputs)
    res = run_bass_kernel_spmd(nc, in_maps, list(range(NCORES)))

    out = np.zeros((T, B, 1, V), np.float32)
    for vq in range(NV):
        o = np.asarray(res.results[vq]["out"]).astype(np.float32)
        out[:, :, 0, VL * vq:VL * vq + VL] = o
    return out


# revision 46
# speedup vs baseline: 1.0005x; 1.0005x over previous
"""Trainium2 Bass kernel for nn_DecoderRNN (LSTM decoder + 32k-vocab projection).

Distribution over 8 NeuronCores:
  - The 23-step LSTM recurrence (B=64) is replicated on every core. Per step a
    fused fp16 weight stream [w | cS | hS | cU | hU | ones] (1602 rows in 13
    128-row chunks) x [lstmS gates | lstmU gates] (2600 cols) accumulates in
    fp32 PSUM. Gate columns are interleaved host-side as [i|f|o|g] x 125
    features per 500-col PSUM bank so one sigmoid covers i,f,o and one tanh
    covers g (2 activation instructions per bank instead of 4).
  - All recurrent state (cS/hS/cU/hU) is produced and kept in fp16: the
    DVE state-update ops write fp16 directly, so the per-step PE transposes
    run at 1 cycle/col instead of fp32's 2 (fp32 transposes were ~14% of
    tensor-engine time) and every state copy moves half the bytes.
  - The vocab projection (61 GFLOP) is sharded 8-way over vocab: each core
    keeps a [626, 4000] bf16 fcW slice (incl. fcb row) resident in SBUF and
    computes [24*64, 626] @ [626, 4000]. The projection is split into 96
    (m,n) units of five 500-col matmuls; units are interleaved into every
    recurrence step behind the bank matmuls but ahead of the (PE) state
    transposes, so the PE streams continuously and its DVFS ramp never
    resets (a cold tensor engine runs at half clock for ~4us).
  - wordOut staging tiles are split per fc m-chunk so a unit's read depends
    only on its own scatter writes, not the latest step's.
  - make_ft (64 batched 25x25 matmuls/step) is software-pipelined one step
    behind the recurrence so its DRAM-bounce regroup DMAs are off the PE
    critical path.
  - Word embeddings are gathered/transposed on the host (indexing only).
"""

import numpy as np
import ml_dtypes

B, T, V, E, D = 64, 24, 32000, 256, 25
H = D * D                       # 625
GCOLS = 4 * H + 4 * D           # 2600
NCORES = 8
NV = 8                          # vocab shards (batch replicated)
VL = V // NV                    # 4000
NM = T // 2                     # 12 fc m-chunks of 128 rows (2 time cols)
NU = VL // 500                  # 8 n-units per m-chunk
NSTEP = T - 1                   # 23
F16, BF16 = np.float16, ml_dtypes.bfloat16

# x-row layout: every block 32-aligned within its 128-chunk (pad rows carry
# zero weights, pad state columns are zeroed)
XW = 0            # 256 word rows
XCS = 256         # 625 cS rows, padded to 640
XHS = 896         # 625 hS rows, padded to 640
XCU = 1536        # 25 cU rows, padded to 32
XHU = 1568        # 25 hU rows, padded to 32
XONE = 1600       # 2 ones rows
XROWS = 1602
NCHUNK = 13       # 12*128 + 66

_COMPILED = {}


def _chunk_rows(c):
    return 128 if c < NCHUNK - 1 else XROWS - 128 * (NCHUNK - 1)


def _win(start, end):
    out, p = [], start
    while p < end:
        n = min(end - p, 128 * (p // 128 + 1) - p)
        out.append((p, n))
        p += n
    return out


def _build_program():
    import concourse.bass as bass
    import concourse.tile as tile
    from concourse import bacc, mybir
    from contextlib import ExitStack

    f16, f32 = mybir.dt.float16, mybir.dt.float32
    bf16 = mybir.dt.bfloat16
    AF = mybir.ActivationFunctionType

    nc = bacc.Bacc("TRN2", target_bir_lowering=False, debug=False,
                   num_devices=NCORES)

    w_all = nc.dram_tensor("w_all", [XROWS, GCOLS], f16, kind="ExternalInput").ap()
    fcq = nc.dram_tensor("fcq", [H + 1, VL], bf16, kind="ExternalInput").ap()
    wut = nc.dram_tensor("wut", [34, H], f16, kind="ExternalInput").ap()
    feat = nc.dram_tensor("feat", [E + 1, B], f16, kind="ExternalInput").ap()
    szt = nc.dram_tensor("szt", [E + 1, H], f16, kind="ExternalInput").ap()
    wt0 = nc.dram_tensor("wt0", [128, 1536], f16, kind="ExternalInput").ap()
    wt1 = nc.dram_tensor("wt1", [128, 1536], f16, kind="ExternalInput").ap()
    iden = nc.dram_tensor("iden", [128, 128], f16, kind="ExternalInput").ap()
    onecol = nc.dram_tensor("onecol", [64, 64], f16, kind="ExternalInput").ap()
    ones16 = nc.dram_tensor("ones16", [2, 64], f16, kind="ExternalInput").ap()
    onesbf = nc.dram_tensor("onesbf", [1, 768], bf16, kind="ExternalInput").ap()
    zerbf = nc.dram_tensor("zerbf", [126, 64], bf16, kind="ExternalInput").ap()
    out = nc.dram_tensor("out", [T, B, VL], bf16, kind="ExternalOutput").ap()

    with tile.TileContext(nc) as tc, ExitStack() as ctx:
        const = ctx.enter_context(tc.tile_pool(name="const", bufs=1))
        states = ctx.enter_context(tc.tile_pool(name="states", bufs=3))
        gact = ctx.enter_context(tc.tile_pool(name="gact", bufs=1))
        xts_pool = ctx.enter_context(tc.tile_pool(name="xtsp", bufs=3))
        ft_pool = ctx.enter_context(tc.tile_pool(name="ftp", bufs=3))
        ft1_pool = ctx.enter_context(tc.tile_pool(name="ftp1", bufs=1))
        lout = ctx.enter_context(tc.tile_pool(name="loutp", bufs=8))
        dram = ctx.enter_context(tc.tile_pool(name="dram", bufs=4, space="DRAM"))
        gpsum = ctx.enter_context(tc.tile_pool(name="gpsum", bufs=4, space="PSUM"))
        tpsum = ctx.enter_context(tc.tile_pool(name="tpsum", bufs=2, space="PSUM"))
        fcpsum = ctx.enter_context(tc.tile_pool(name="fcps", bufs=2, space="PSUM"))

        # ---------- persistent SBUF (init-critical small tensors first so
        # the sStart matmul + first transposes aren't stuck behind the big
        # weight stream; big loads alternate between the two HWDGE queues) --
        FEAT, SZT = [], []
        for c, r in ((0, 128), (1, 128), (2, 1)):
            tf = const.tile([r, B], f16, tag=f"feat{c}")
            nc.sync.dma_start(tf[:], feat[128 * c:128 * c + r, :])
            FEAT.append(tf)
            ts = const.tile([r, H], f16, tag=f"szt{c}")
            nc.scalar.dma_start(ts[:], szt[128 * c:128 * c + r, :])
            SZT.append(ts)
        IDEN = const.tile([128, 128], f16, tag="iden")
        nc.sync.dma_start(IDEN[:], iden)
        ONECOL = const.tile([64, 64], f16, tag="onecol")
        nc.sync.dma_start(ONECOL[:], onecol)
        ONES16 = const.tile([2, 64], f16, tag="ones16")
        nc.sync.dma_start(ONES16[:], ones16)
        WUT = const.tile([34, H], f16, tag="wut")
        nc.scalar.dma_start(WUT[:], wut)
        WT0 = const.tile([128, 1536], f16, tag="wt0")
        nc.sync.dma_start(WT0[:], wt0)
        WT1 = const.tile([128, 1536], f16, tag="wt1")
        nc.scalar.dma_start(WT1[:], wt1)
        WA = []
        for c in range(NCHUNK):
            r = _chunk_rows(c)
            t_ = const.tile([r, GCOLS], f16, tag=f"wa{c}")
            eng = nc.sync if c % 2 == 0 else nc.scalar
            eng.dma_start(t_[:], w_all[128 * c:128 * c + r, :])
            WA.append(t_)
        F16Z = const.tile([64, 16], f16, tag="f16z")
        nc.vector.memset(F16Z[:], 0.0)

        # wordOut staging, one tile set per fc m-chunk (time cols 2m, 2m+1)
        WO = []
        for m in range(NM):
            row = []
            for c in range(5):
                r = 126 if c == 4 else 125
                t_ = const.tile([r, 128], bf16, tag=f"wo{m}_{c}")
                if m == 0:
                    nc.sync.dma_start(t_[:, 0:B], zerbf[0:r, :])
                row.append(t_)
            WO.append(row)
        for m in range(NM):
            nc.sync.dma_start(WO[m][4][125:126, :], onesbf[:, 0:128])
        FC = []
        for c in range(5):
            r = 126 if c == 4 else 125
            t_ = const.tile([r, VL], bf16, tag=f"fc{c}")
            eng = nc.sync if c % 2 == 0 else nc.scalar
            eng.dma_start(t_[:], fcq[125 * c:125 * c + r, :])
            FC.append(t_)

        # ---------- helpers ----------
        def xts_ap(xts, xr0, n):
            c = xr0 // 128
            assert (xr0 + n - 1) // 128 == c and c >= 2
            return xts[xr0 % 128:xr0 % 128 + n,
                       64 * (c - 2):64 * (c - 2) + 64]

        def transpose_into(src_, xts, xr0, total):
            col = 0
            for (p0, n) in _win(xr0, xr0 + total):
                ps = tpsum.tile([128, B], f16, tag="tp")
                nc.tensor.transpose(ps[0:n, :], src_[:, col:col + n],
                                    IDEN[0:B, 0:B])
                nc.vector.tensor_copy(xts_ap(xts, p0, n), ps[0:n, :])
                col += n

        BANKS = [(0, 500), (500, 500), (1000, 500), (1500, 500),
                 (2000, 500), (2500, 100)]

        # fc work split into (m, n) units of 5 accumulating 500-col matmuls
        # each (~1us at full clock), emitted singly into recurrence stall
        # slots. m-chunk m (time cols 2m, 2m+1) needs ft(tcol 2m+1) which is
        # scattered by emit_ft_matmuls(2m) during step 2m+2 -> legal s>=2m+3.
        fc_cursor = [0]
        NPAIR = NU // 2

        def emit_fc_units(k):
            # pair-units: the WO stationary is reused across two 500-col
            # matmuls so its LDWEIGHTS amortizes
            for _ in range(k):
                u = fc_cursor[0]
                if u >= NM * NPAIR:
                    return
                fc_cursor[0] += 1
                m, pr = u // NPAIR, u % NPAIR
                fp0 = fcpsum.tile([128, 500], f32, tag="fcp")
                fp1 = fcpsum.tile([128, 500], f32, tag="fcp")
                fps = [fp0, fp1]
                for c in range(5):
                    for j in range(2):
                        n = 2 * pr + j
                        nc.tensor.matmul(
                            fps[j][:], WO[m][c][:],
                            FC[c][:, 500 * n:500 * n + 500],
                            start=(c == 0), stop=(c == 4),
                            skip_group_check=True)
                for j in range(2):
                    n = 2 * pr + j
                    lo = lout.tile([128, 500], bf16, tag="lo")
                    if j == 0:
                        nc.vector.tensor_copy(lo[:], fps[j][:])
                    else:
                        nc.scalar.activation(lo[:], fps[j][:], AF.Copy)
                    nc.sync.dma_start(
                        out[2 * m:2 * m + 2, :, 500 * n:500 * n + 500], lo[:])

        # ---------- initial state (all recurrent state is fp16) ----------
        CS = states.tile([B, 640], f16, tag="cs")
        sp0 = gpsum.tile([B, 500], f32, tag="gp")
        sp1 = gpsum.tile([B, 500], f32, tag="gp")
        for ci in range(3):
            nc.tensor.matmul(sp0[:, 0:500], FEAT[ci][:], SZT[ci][:, 0:500],
                             start=(ci == 0), stop=(ci == 2))
            nc.tensor.matmul(sp1[:, 0:125], FEAT[ci][:], SZT[ci][:, 500:625],
                             start=(ci == 0), stop=(ci == 2))
        nc.vector.tensor_copy(CS[:, 0:500], sp0[:, 0:500])
        nc.vector.tensor_copy(CS[:, 500:625], sp1[:, 0:125])
        nc.vector.tensor_copy(CS[:, 625:640], F16Z[:, 0:15])
        CU = states.tile([B, 32], f16, tag="cu")
        nc.vector.memset(CU[:], 0.0)

        XTS = xts_pool.tile([128, 11 * 64], f16, tag="xts")
        nc.vector.memset(XTS[:], 0.0)
        nc.vector.tensor_copy(xts_ap(XTS, XONE, 2), ONES16[:])
        transpose_into(CS, XTS, XCS, 640)

        # make_ft tiles carried across two steps (software pipelining deep
        # enough that the DRAM-bounce regroup DMAs always land early)
        pipe = []

        def emit_ft_matmuls(s):
            # consume UTT/M2T (DMA'd during step s) -> TTS -> ttd -> WO col s+1
            UTT, M2T = pipe.pop(0)
            TTS = ft1_pool.tile([D, B * D], f32, tag="tts")
            for q in range(4):
                ttf = gpsum.tile([B, 500], f32, tag="gp")
                tt = ttf[0:D, 0:16 * D]
                for bl in range(16):
                    b = 16 * q + bl
                    nc.tensor.matmul(tt[:, D * bl:D * bl + D],
                                     M2T[:, D * b:D * b + D],
                                     UTT[:, D * b:D * b + D],
                                     start=True, stop=True)
                tdst = (TTS[:].rearrange("j (i b) -> j i b", i=D)
                        [:, :, 16 * q:16 * q + 16])
                tsrc = tt[:].rearrange("j (b i) -> j i b", b=16)
                if q % 2 == 0:
                    nc.vector.tensor_copy(tdst, tsrc)
                else:
                    nc.scalar.activation(tdst, tsrc, AF.Copy)
            ttd = dram.tile([D, B * D], f32, tag="ttd")
            nc.sync.dma_start(ttd[:], TTS[:])
            tcol = s + 1
            wm, wl = tcol // 2, tcol % 2
            for c in range(5):
                nc.gpsimd.dma_start(
                    WO[wm][c][0:125, B * wl:B * wl + B],
                    bass.AP(ttd.tensor, 5 * c * B * D,
                            [[B * D, 5], [B, D], [1, B]]))

        # ---------- recurrence ----------
        for s in range(NSTEP):
            def xchunk(c, _xts=XTS, _s=s):
                if c == 0:
                    return WT0[:, 64 * _s:64 * _s + 64]
                if c == 1:
                    return WT1[:, 64 * _s:64 * _s + 64]
                r = _chunk_rows(c)
                return _xts[0:r, 64 * (c - 2):64 * (c - 2) + 64]

            gps = []
            for (col0, n) in BANKS:
                gp = gpsum.tile([B, 500], f32, tag="gp")
                # the U-gate bank (cols 2500:) has structurally zero weights
                # for the hS row chunks 7..11 — skip those matmuls
                chunks = (list(range(NCHUNK)) if col0 < 4 * H
                          else [0, 1, 2, 3, 4, 5, 6, 12])
                for j, c in enumerate(chunks):
                    nc.tensor.matmul(gp[:, 0:n], xchunk(c),
                                     WA[c][:, col0:col0 + n],
                                     start=(j == 0), stop=(j == len(chunks) - 1))
                gps.append(gp)

            # PE fill work while the step-s activation/update tail runs
            if s >= 2:
                emit_ft_matmuls(s - 2)
            if s == NSTEP - 1:
                # last step: also consume the s-1 pipe entry now (its regroup
                # DMAs landed during this step's banks) so its matmuls fill
                # this step's tail and its scatter completes before the drain
                emit_ft_matmuls(s - 1)

            # fc units behind the banks but ahead of the (PE) transposes:
            # they stream while Scalar/Vector run this step's act+update, so
            # the PE never idles and its DVFS ramp never resets.
            allowed = (NPAIR * ((s - 3) // 2 + 1)) if s >= 3 else 0
            emit_fc_units(min(max(allowed - fc_cursor[0], 0), 2))

            # per-bank [i|f|o|g] x 125-feature activation + state update,
            # cS'/hS' written directly as fp16
            CSn = states.tile([B, 640], f16, tag="cs")
            HSn = states.tile([B, 640], f16, tag="hs")
            TC = gact.tile([B, H], f32, tag="tc")
            for b in range(5):
                fsl = slice(125 * b, 125 * b + 125)
                sio = gact.tile([B, 375], f32, tag="gsio")
                gb = gact.tile([B, 125], f32, tag="ggb")
                nc.scalar.activation(sio[:], gps[b][:, 0:375], AF.Sigmoid)
                nc.scalar.activation(gb[:], gps[b][:, 375:500], AF.Tanh)
                t1 = gact.tile([B, 125], f32, tag="t1")
                t2 = gact.tile([B, 125], f32, tag="t2")
                nc.vector.tensor_mul(t1[:], sio[:, 125:250], CS[:, fsl])
                nc.vector.tensor_mul(t2[:], sio[:, 0:125], gb[:])
                nc.vector.tensor_add(CSn[:, fsl], t1[:], t2[:])
                nc.scalar.activation(TC[:, fsl], CSn[:, fsl], AF.Tanh)
                nc.vector.tensor_mul(HSn[:, fsl], sio[:, 250:375], TC[:, fsl])
            nc.vector.tensor_copy(CSn[:, H:640], F16Z[:, 0:15])
            nc.vector.tensor_copy(HSn[:, H:640], F16Z[:, 0:15])

            # U-LSTM (bank 5), gates laid out [i|f|o|g] x 25
            usio = gact.tile([B, 75], f32, tag="usio")
            ug = gact.tile([B, D], f32, tag="ug")
            nc.scalar.activation(usio[:], gps[5][:, 0:75], AF.Sigmoid)
            nc.scalar.activation(ug[:], gps[5][:, 75:100], AF.Tanh)
            CUn = states.tile([B, 32], f16, tag="cu")
            t1u = gact.tile([B, D], f32, tag="t1u")
            t2u = gact.tile([B, D], f32, tag="t2u")
            nc.vector.tensor_mul(t1u[:], usio[:, 25:50], CU[:, 0:D])
            nc.vector.tensor_mul(t2u[:], usio[:, 0:25], ug[:])
            nc.vector.tensor_add(CUn[:, 0:D], t1u[:], t2u[:])
            nc.vector.tensor_copy(CUn[:, D:32], F16Z[:, 0:7])
            TCU = gact.tile([B, D], f32, tag="tcu")
            nc.scalar.activation(TCU[:], CUn[:, 0:D], AF.Tanh)
            HUn = states.tile([B, 32], f16, tag="hu")
            nc.vector.tensor_mul(HUn[:, 0:D], usio[:, 50:75], TCU[:])
            nc.vector.tensor_copy(HUn[:, D:32], F16Z[:, 0:7])

            if s < NSTEP - 1:
                XTSn = xts_pool.tile([128, 11 * 64], f16, tag="xts")
                nc.vector.tensor_copy(xts_ap(XTSn, XONE, 2), ONES16[:])
                transpose_into(CSn, XTSn, XCS, 640)
                transpose_into(HSn, XTSn, XHS, 640)
            else:
                XTSn = None
            hups = tpsum.tile([128, B], f16, tag="tp")
            nc.tensor.transpose(hups[0:32, :], HUn[:], IDEN[0:B, 0:B])
            if XTSn is not None:
                transpose_into(CUn, XTSn, XCU, 32)
                nc.vector.tensor_copy(xts_ap(XTSn, XHU, 32), hups[0:32, :])
            HUTn = states.tile([34, B], f16, tag="hut")
            nc.vector.tensor_copy(HUTn[0:32, :], hups[0:32, :])
            nc.vector.tensor_copy(HUTn[32:34, :], ONES16[:])

            up0 = gpsum.tile([B, 500], f32, tag="gp")
            up1 = gpsum.tile([B, 500], f32, tag="gp")
            nc.tensor.matmul(up0[:, 0:500], HUTn[:], WUT[:, 0:500],
                             start=True, stop=True)
            nc.tensor.matmul(up1[:, 0:125], HUTn[:], WUT[:, 500:625],
                             start=True, stop=True)
            UT = ft1_pool.tile([B, H], f16, tag="ut")
            nc.vector.tensor_copy(UT[:, 0:500], up0[:, 0:500])
            nc.vector.tensor_copy(UT[:, 500:625], up1[:, 0:125])

            # stage make_ft inputs for all 64 batch rows via DRAM-bounce
            # regroup; consumed by emit_ft_matmuls at step s+2
            utd = dram.tile([B, H], f16, tag="utd")
            nc.sync.dma_start(utd[:], UT[:])
            m2d = dram.tile([B, H], f16, tag="m2d")
            nc.gpsimd.dma_start(m2d[:], HSn[:, 0:H])
            UTT = ft_pool.tile([D, B * D], f16, tag="utt")
            nc.sync.dma_start(
                UTT[:], bass.AP(utd.tensor, 0, [[D, D], [H, B], [1, D]]))
            M2T = ft_pool.tile([D, B * D], f16, tag="m2t")
            nc.sync.dma_start(
                M2T[:], bass.AP(m2d.tensor, 0, [[D, D], [H, B], [1, D]]))
            pipe.append((UTT, M2T))

            CS, CU, XTS = CSn, CUn, XTSn

            # one more unit at step end to cover the XTS-copy dependency of
            # the next step's first bank matmul
            emit_fc_units(min(max(allowed - fc_cursor[0], 0), 1))

        # drain: m=10 units are legal (ft(21)'s scatter ran inside step 22)
        # and overlap ft(22)'s regroup wait in the PE queue
        emit_fc_units(min(max(NPAIR * (NM - 1) - fc_cursor[0], 0), NPAIR))
        emit_ft_matmuls(NSTEP - 1)
        emit_fc_units(NM * NPAIR - fc_cursor[0])

    nc.compile()
    return nc


# interleaved S-gate column permutation: bank b (0..4) holds
# [i|f|o|g] x features 125b..125b+125
def _gate_perm():
    idx = np.empty(2500, np.int64)
    for p, g in enumerate((0, 1, 3, 2)):
        for b in range(5):
            idx[500 * b + 125 * p:500 * b + 125 * p + 125] = \
                625 * g + 125 * b + np.arange(125)
    return idx


# U gate columns reordered [i|f|o|g] x 25
def _ugate_perm():
    return np.concatenate([np.arange(25), 25 + np.arange(25),
                           75 + np.arange(25), 50 + np.arange(25)])


def _host_prep(inputs):
    f32 = lambda k: np.asarray(inputs[k], dtype=np.float32)
    features = f32("features")
    captions = np.asarray(inputs["captions"]).astype(np.int64)
    embed = f32("embed_table")
    WihS, WhhS = f32("WihS"), f32("WhhS")
    bihS, bhhS = f32("bihS"), f32("bhhS")
    WihU, WhhU = f32("WihU"), f32("WhhU")
    bihU, bhhU = f32("bihU"), f32("bhhU")
    fcW, fcb = f32("fcW"), f32("fcb")
    szW, szb = f32("szW"), f32("szb")
    wuW, wub = f32("wuW"), f32("wub")

    w_all = np.zeros((XROWS, GCOLS), np.float32)
    WihS_T, WihU_T = WihS.T, WihU.T
    w_all[XW:XW + 256, :2500] = WihS_T[25:281]
    w_all[XW:XW + 256, 2500:] = WihU_T[25:281]
    w_all[XCS:XCS + 625, :2500] = WihS_T[281:906]
    w_all[XCS:XCS + 625, 2500:] = WihU_T[281:906]
    w_all[XHS:XHS + 625, :2500] = WhhS.T
    w_all[XCU:XCU + 25, :2500] = WihS_T[0:25]
    w_all[XCU:XCU + 25, 2500:] = WihU_T[0:25]
    w_all[XHU:XHU + 25, 2500:] = WhhU.T
    w_all[XONE, :2500] = bihS + bhhS
    w_all[XONE, 2500:] = bihU + bhhU
    w_all[:, :2500] = w_all[:, _gate_perm()]
    w_all[:, 2500:] = w_all[:, 2500 + _ugate_perm()]
    w_all = np.ascontiguousarray(w_all).astype(F16)

    fcW_perm = np.ascontiguousarray(
        fcW.reshape(V, D, D).transpose(0, 2, 1).reshape(V, H))
    wuW_perm = np.ascontiguousarray(
        wuW.reshape(D, D, D).transpose(1, 0, 2).reshape(H, D))
    wub_perm = np.ascontiguousarray(wub.reshape(D, D).T.reshape(H))
    wut = np.zeros((34, H), np.float32)
    wut[0:25] = wuW_perm.T
    wut[32] = wub_perm
    wut = wut.astype(F16)

    szt = np.concatenate([szW.T, szb[None, :]], 0).astype(F16)
    emb16 = embed.astype(F16)
    iden = np.eye(128, dtype=F16)
    onecol = np.zeros((64, 64), F16)
    onecol[:, 0] = 1.0

    feat_r = np.concatenate([features.T,
                             np.ones((1, B), np.float32)], 0).astype(F16)
    # host-side embedding gather+transpose: wt[e, 64*s+b] = emb[cap[b,s], e]
    # slot 0 is the word0 start token = embed_table[0] for every batch row
    wt = np.zeros((E, 1536), F16)
    wt[:, 0:64] = emb16[0][:, None]
    for s in range(1, 23):
        wt[:, 64 * s:64 * s + 64] = emb16[captions[:, s]].T

    in_maps = []
    for vq in range(NV):
        fcq = np.concatenate(
            [fcW_perm.T[:, VL * vq:VL * vq + VL],
             fcb[None, VL * vq:VL * vq + VL]], 0).astype(BF16)
        in_maps.append({
            "w_all": w_all, "fcq": np.ascontiguousarray(fcq),
            "wut": wut, "feat": feat_r, "szt": szt,
            "wt0": np.ascontiguousarray(wt[0:128]),
            "wt1": np.ascontiguousarray(wt[128:256]),
            "iden": iden, "onecol": onecol,
            "ones16": np.ones((2, 64), F16),
            "onesbf": np.ones((1, 768), BF16),
            "zerbf": np.zeros((126, 64), BF16),
        })
    return in_maps


def kernel(**inputs):
    from concourse.bass_utils import run_bass_kernel_spmd

    if "prog" not in _COMPILED:
        _COMPILED["prog"] = _build_program()
    nc = _COMPILED["prog"]

    in_maps = _host_prep(inputs)
    res = run_bass_kernel_spmd(nc, in_maps, list(range(NCORES)))

    out = np.zeros((T, B, 1, V), np.float32)
    for vq in range(NV):
        o = np.asarray(res.results[vq]["out"]).astype(np.float32)
        out[:, :, 0, VL * vq:VL * vq + VL] = o
    return out
